# revision 6
# baseline (speedup 1.0000x reference)
"""Trainium2 Bass kernel for nn_DistDistance (retrieval_knn).

Reference computation (per batch b):
    dist2[l2, l1] = || c1[b,l1,:] - c2[b,l2,:] + eps ||^2
    out[b] = mean_l2( sqrt(max(min_l1 dist2, 0)) * resolution )

Device strategy (8 cores, SPMD): core c handles batch b=c//2, query half
h=c%2 (2048 of the 4096 l2 points; 16 tiles of 128 queries on the
partition axis).

Pair-min via PE accumulation (the key structure):
    min(a, b) = -( relu(a - b) - a )
  For each l1 pair (i, j=i+2048) and query q:
    - mm_T (PE):  Y[q, m] = d_i - d_j       (difference columns, K=18 matmul)
    - ACT:        X = Relu(Y)               (PSUM -> PSUM, 1 elem/cycle)
    - mm_S (PE):  X += -d_i                 (start=False accumulate onto the
                                             ACT-written bank)
      => X[q, m] = -min(d_i, d_j) exactly in fp32 PSUM.
    - DVE:        tensor_reduce(max) over X -> -min over all l1 per query.
  This halves the values the reduce crew must process and eliminates the
  fp16 tournament entirely: ACT and DVE are both pure single-pass drains.

dist2 is produced directly (not a bracket): the per-query t2 = |c2_q|^2 term
rides the matmul as 3 extra bf16-split rows against a +-1 u-row, so no bias
add is needed anywhere. All fp32 values are split hi+lo into bf16 so the PE
streams 1 cycle/row; products of bf16 splits are exact.

PSUM: Y split into two [128,1024] slabs (2 banks each) and X likewise;
bufs=1 pools give exactly the pipeline: Y(t+1) refills as soon as ACT(t)
read it, X(t+1) refills as soon as DVE(t) drained it, half-tile granular.
The two X-half maxes go to separate output columns; the host merges them
(free), so there is no combine op on the device.
"""

import numpy as np
import ml_dtypes
from contextlib import ExitStack

import concourse.mybir as mybir
import concourse.tile as tile
from concourse import bacc, bass_utils
from bass_rust import add_dep_helper

B, L1, L2, D = 4, 4096, 4096, 3
EPS = 1e-6
NCORES = 8
L2H = L2 // 2        # l2 points per core
NT = L2H // 128      # l2 tiles per core
NPAIR = L1 // 2      # pair-min columns per tile
K = 18               # contraction rows (12 cross + 3 a1-split + 3 t2-split)
HALF = NPAIR // 2    # columns per X/Y half-slab (2 PSUM banks)

BF16 = ml_dtypes.bfloat16

FLUSH_EVERY = 4      # out-DMA granularity (tiles)


def _build_nc(reps=1):
    nc = bacc.Bacc("TRN2", num_devices=NCORES)
    vt = nc.dram_tensor("vt", [128, L2H], mybir.dt.bfloat16, kind="ExternalInput").ap()
    utT = nc.dram_tensor("utT", [128, NPAIR], mybir.dt.bfloat16, kind="ExternalInput").ap()
    utS = nc.dram_tensor("utS", [128, NPAIR], mybir.dt.bfloat16, kind="ExternalInput").ap()
    # out[:, 0:NT] / out[:, NT:2*NT]: -min over each X half; host merges.
    out = nc.dram_tensor("out", [128, 2 * NT], mybir.dt.float32, kind="ExternalOutput").ap()

    with tile.TileContext(nc) as tc, ExitStack() as ctx:
        const = ctx.enter_context(tc.tile_pool(name="const", bufs=1))
        psY = ctx.enter_context(tc.tile_pool(name="psY", bufs=1, space="PSUM"))
        psX = ctx.enter_context(tc.tile_pool(name="psX", bufs=1, space="PSUM"))
        stats = ctx.enter_context(tc.tile_pool(name="stats", bufs=1))
        small = ctx.enter_context(tc.tile_pool(name="small", bufs=2))

        utT_sb = const.tile([128, NPAIR], mybir.dt.bfloat16)
        utS_sb = const.tile([128, NPAIR], mybir.dt.bfloat16)
        vt_sb = const.tile([128, L2H], mybir.dt.bfloat16)

        # Warm the ACT Relu table before any real work (overlaps input DMA).
        warm = small.tile([128, 1], mybir.dt.float32, tag="warm")
        nc.vector.memset(warm, 0.0)
        nc.scalar.activation(warm, warm, mybir.ActivationFunctionType.Relu)

        # Input DMA: only the 4 quadrant stripes of 18 rows carry data.
        # Tile-0's matmuls need utT/utS cols [0:512] per stripe first, then
        # the rest; vt streams in tile order on a third queue.
        for j in range(4):
            nc.sync.dma_start(
                out=utT_sb[32 * j : 32 * j + K, 0:512],
                in_=utT[32 * j : 32 * j + K, 0:512],
            )
            nc.gpsimd.dma_start(
                out=utS_sb[32 * j : 32 * j + K, 0:512],
                in_=utS[32 * j : 32 * j + K, 0:512],
            )
        for j in range(4):
            nc.sync.dma_start(
                out=utT_sb[32 * j : 32 * j + K, 512:NPAIR],
                in_=utT[32 * j : 32 * j + K, 512:NPAIR],
            )
            nc.gpsimd.dma_start(
                out=utS_sb[32 * j : 32 * j + K, 512:NPAIR],
                in_=utS[32 * j : 32 * j + K, 512:NPAIR],
            )
        VBLK = 256
        for blk in range(L2H // VBLK):
            for j in range(4):
                nc.scalar.dma_start(
                    out=vt_sb[32 * j : 32 * j + K, blk * VBLK : (blk + 1) * VBLK],
                    in_=vt[32 * j : 32 * j + K, blk * VBLK : (blk + 1) * VBLK],
                )

        mins = stats.tile([128, 2 * NT], mybir.dt.float32)

        def emit_tile(t):
            q0 = t * 128
            for h in range(2):  # half-tile: 1024 pair columns, 2 PSUM banks
                # Quadrants 2h+c: each 512-chunk streams in its own PE
                # row-group and PSUM bank, so the two chunks of a half (and
                # the other half's) can overlap in the array.
                Y = psY.tile([128, HALF], mybir.dt.float32, tag=f"Y{h}")
                for c in range(2):
                    col0 = h * HALF + c * 512
                    bp = 32 * (2 * h + c)
                    nc.tensor.matmul(
                        Y[:, c * 512 : (c + 1) * 512],
                        lhsT=vt_sb[bp : bp + K, q0 : q0 + 128],
                        rhs=utT_sb[bp : bp + K, col0 : col0 + 512],
                        start=True,
                        stop=True,
                        tile_position=(bp, 0),
                    )
                X = psX.tile([128, HALF], mybir.dt.float32, tag=f"X{h}")
                relu = nc.scalar.activation(
                    X, Y, mybir.ActivationFunctionType.Relu
                )
                for c in range(2):
                    col0 = h * HALF + c * 512
                    bp = 32 * (2 * h + c)
                    mm = nc.tensor.matmul(
                        X[:, c * 512 : (c + 1) * 512],
                        lhsT=vt_sb[bp : bp + K, q0 : q0 + 128],
                        rhs=utS_sb[bp : bp + K, col0 : col0 + 512],
                        start=False,
                        stop=True,
                        tile_position=(bp, 0),
                    )
                    # start=False is a PSUM read-modify-write: the scheduler
                    # only sees a WAW on X, which does not order the PE's
                    # accumulate after the ACT write. Force the RAW edge.
                    add_dep_helper(
                        mm.ins, relu.ins, sync=True,
                        reason="mm_S accumulates onto ACT-written relu bank",
                    )
                nc.vector.tensor_reduce(
                    mins[:, h * NT + t : h * NT + t + 1],
                    X,
                    axis=mybir.AxisListType.X,
                    op=mybir.AluOpType.max,
                )
            if t % FLUSH_EVERY == FLUSH_EVERY - 1:
                lo = t - FLUSH_EVERY + 1
                nc.sync.dma_start(
                    out=out[:, lo : t + 1], in_=mins[:, lo : t + 1]
                )
                nc.sync.dma_start(
                    out=out[:, NT + lo : NT + t + 1],
                    in_=mins[:, NT + lo : NT + t + 1],
                )

        def body():
            for t in range(NT):
                emit_tile(t)

        if reps == 1:
            body()
        else:
            with tc.For_i(0, reps, 1):
                body()

    nc.finalize()
    return nc


def _split2(x):
    hi = x.astype(BF16)
    lo = (x - hi.astype(np.float32)).astype(BF16)
    return hi, lo


def _split3(x):
    p = x.astype(BF16)
    r1 = x - p.astype(np.float32)
    q = r1.astype(BF16)
    r = (r1 - q.astype(np.float32)).astype(BF16)
    return p, q, r


def _urows(coord, a1):
    """Build the 18 u-side rows for columns with 3-vector coordinate content
    `coord` [3, N] (fp32) and scalar row content a1 [N] (fp32): rows pair with
    v-rows [hw, hw, lw, lw, ones, t2-splits]. The t2-partner rows are filled
    by the caller."""
    h, lo = _split2(coord)
    p, q, r = _split3(a1[None, :])
    return np.concatenate([h, lo, h, lo, p, q, r], axis=0)  # [15, N]


def _prep_core(c1b, c2b, h):
    """Build vt/utT/utS operands for one core (batch data c1b/c2b, l2 half h)."""
    c2h = c2b[h * L2H : (h + 1) * L2H]  # [L2H, 3]

    # Effective (bf16-split-representable) contour1 coordinates; all scalar
    # rows (a1) must be derived from these so the expansion cancels exactly.
    h1, l1 = _split2(c1b.T)                                   # [3, L1]
    c1eff = h1.astype(np.float64) + l1.astype(np.float64)     # [3, L1]
    s1 = np.sum(c1eff * c1eff, axis=0)
    sum1 = np.sum(c1eff, axis=0)
    a1 = (s1 + 2.0 * EPS * sum1).astype(np.float32)           # [L1]
    c1f = c1eff.astype(np.float32)

    # Query side: w = -2*c2 split hi/lo; effective c2 from the splits.
    w = -2.0 * c2h.T                                          # [3, L2H]
    hw, lw = _split2(w)
    c2eff = -(hw.astype(np.float64) + lw.astype(np.float64)) / 2.0
    s2 = np.sum(c2eff * c2eff, axis=0)
    sum2 = np.sum(c2eff, axis=0)
    t2 = (s2 - 2.0 * EPS * sum2 + D * EPS * EPS).astype(np.float32)  # [L2H]
    p2, q2, r2 = _split3(t2[None, :])
    vrows = np.concatenate(
        [hw, hw, lw, lw,
         np.ones((3, L2H), dtype=BF16),
         p2, q2, r2], axis=0,
    )  # [21, L2H] -> rows 12:15 are the a1 partners (ones), 15:18 t2 partners
    # NOTE row layout must match _urows: [hw,hw,lw,lw]=12 rows, then 3 ones
    # rows pairing a1 splits p,q,r, then 3 t2-split rows pairing +-1/0 u-rows.
    assert vrows.shape[0] == K

    # T columns: pair (i, i+NPAIR): content = effective difference, no t2 row.
    coordT = (c1f[:, :NPAIR] - c1f[:, NPAIR:]).astype(np.float32)
    a1T = a1[:NPAIR] - a1[NPAIR:]
    uT = np.concatenate(
        [_urows(coordT, a1T), np.zeros((3, NPAIR), dtype=BF16)], axis=0
    )  # [18, NPAIR]

    # S columns: -d_i -> negate coords, a1, and t2 partner (-1).
    uS = np.concatenate(
        [_urows(-c1f[:, :NPAIR], -a1[:NPAIR]),
         np.full((3, NPAIR), -1.0, dtype=BF16)], axis=0
    )  # [18, NPAIR]

    vt = np.zeros((128, L2H), dtype=BF16)
    utT = np.zeros((128, NPAIR), dtype=BF16)
    utS = np.zeros((128, NPAIR), dtype=BF16)
    for j in range(4):
        vt[32 * j : 32 * j + K, :] = vrows
        utT[32 * j : 32 * j + K, :] = uT
        utS[32 * j : 32 * j + K, :] = uS
    return vt, utT, utS


_NC_CACHE = []


def _get_nc():
    if not _NC_CACHE:
        _NC_CACHE.append(_build_nc())
    return _NC_CACHE[0]


def kernel(contour1, contour2, resolution):
    c1 = np.asarray(contour1, dtype=np.float32)
    c2 = np.asarray(contour2, dtype=np.float32)
    res = float(np.asarray(resolution).reshape(-1)[0])

    in_maps = []
    for core in range(NCORES):
        b, h = core // 2, core % 2
        vt, utT, utS = _prep_core(c1[b], c2[b], h)
        in_maps.append({"vt": vt, "utT": utT, "utS": utS})

    nc = _get_nc()
    results = bass_utils.run_bass_kernel_spmd(
        nc, in_maps, core_ids=list(range(NCORES))
    ).results

    out = np.empty((B,), dtype=np.float32)
    for b in range(B):
        halves = []
        for h in range(2):
            r = results[2 * b + h]["out"]  # [128, 2*NT]
            neg_min = np.maximum(r[:, :NT], r[:, NT:])  # [p, t] -> -min dist2
            d2 = -neg_min.T.reshape(L2H)
            halves.append(np.sqrt(np.maximum(d2, 0.0)))
        min_dist = np.concatenate(halves)
        out[b] = np.float32(np.mean(min_dist * res))
    return out


# revision 8
# speedup vs baseline: 1.2960x; 1.2960x over previous
"""Trainium2 Bass kernel for nn_DistDistance (retrieval_knn).

Reference computation (per batch b):
    dist2[l2, l1] = || c1[b,l1,:] - c2[b,l2,:] + eps ||^2
    out[b] = mean_l2( sqrt(max(min_l1 dist2, 0)) * resolution )

Device strategy (8 cores, SPMD): core c handles batch b=c//2, query half
h=c%2 (2048 of the 4096 l2 points; 16 tiles of 128 queries on the
partition axis).

Pair-min via PE accumulation (the key structure):
    -min(a, b) = relu(a - b) - a
  For each l1 pair (i, j=i+2048) and query q:
    - mm_T (PE):   P[q, m] = d_i - d_j      (difference columns, K=18 bf16)
    - ACT:         P = Relu(P)              (in-place PSUM -> PSUM pass)
    - mm_S (PE):   P += -d_i                (start=False accumulate onto the
                                             ACT-written bank)
      => P[q, m] = -min(d_i, d_j) exactly in fp32 PSUM.
    - DVE:         tensor_reduce(max) over P -> -min over l1 per query.
  This halves the values the reduce crew processes and removes the fp16
  tournament entirely: ACT and DVE are both pure single-pass drains
  (ACT 1707ns + DVE 2133ns of work per tile instead of ~3200ns each).

dist2 is produced directly: the per-query t2 = |c2_q|^2 rides the matmul as
3 extra bf16-split v-rows against a -1/0 u-row, so no bias add anywhere.
All fp32 operands split hi+lo into bf16 (products of bf16 are exact).

PSUM: slabs are [128, 1024] (2 banks); one pool, 4 slots. The in-place
relu means each slab serves as both matmul dest and accumulate dest, so 4
slabs cover 2 tiles in flight: relu(t) overlaps reduce(t-1) on other slots.
The PE's start=False accumulate is a read-modify-write the dep tracker sees
only as WAW, so an explicit dep orders it after the relu.

The per-half maxes land in separate mins columns; the host does the final
2-way max + sqrt + mean (negligible).
"""

import numpy as np
import ml_dtypes
from contextlib import ExitStack

import concourse.mybir as mybir
import concourse.tile as tile
from concourse import bacc, bass_utils
from bass_rust import add_dep_helper

B, L1, L2, D = 4, 4096, 4096, 3
EPS = 1e-6
NCORES = 8
L2H = L2 // 2        # l2 points per core
NT = L2H // 128      # l2 tiles per core
NPAIR = L1 // 2      # pair-min columns per tile
K = 18               # contraction rows (12 cross + 3 a1-split + 3 t2-split)
HALF = NPAIR // 2    # columns per slab (2 PSUM banks)
NSLAB = 2 * NT       # slabs total (2 per tile)

BF16 = ml_dtypes.bfloat16

FLUSH_EVERY = 8      # out-DMA granularity (slabs)


def _build_nc(reps=1):
    nc = bacc.Bacc("TRN2", num_devices=NCORES)
    vt = nc.dram_tensor("vt", [128, L2H], mybir.dt.bfloat16, kind="ExternalInput").ap()
    utT = nc.dram_tensor("utT", [128, NPAIR], mybir.dt.bfloat16, kind="ExternalInput").ap()
    utS = nc.dram_tensor("utS", [128, NPAIR], mybir.dt.bfloat16, kind="ExternalInput").ap()
    # out[:, s]: -min over slab s = (t, h) = (s//2, s%2); host merges pairs.
    out = nc.dram_tensor("out", [128, NSLAB], mybir.dt.float32, kind="ExternalOutput").ap()

    with tile.TileContext(nc) as tc, ExitStack() as ctx:
        const = ctx.enter_context(tc.tile_pool(name="const", bufs=1))
        psum = ctx.enter_context(tc.tile_pool(name="psum", bufs=1, space="PSUM"))
        stats = ctx.enter_context(tc.tile_pool(name="stats", bufs=1))
        small = ctx.enter_context(tc.tile_pool(name="small", bufs=2))

        utT_sb = const.tile([128, NPAIR], mybir.dt.bfloat16)
        utS_sb = const.tile([128, NPAIR], mybir.dt.bfloat16)
        vt_sb = const.tile([128, L2H], mybir.dt.bfloat16)

        # Warm the ACT Relu table before any real work (overlaps input DMA).
        warm = small.tile([128, 1], mybir.dt.float32, tag="warm")
        nc.vector.memset(warm, 0.0)
        nc.scalar.activation(warm, warm, mybir.ActivationFunctionType.Relu)

        # Input DMA: only the 4 quadrant stripes of 18 rows carry data.
        # Slab 0/1 need utT/utS cols [0:1024] first; vt streams in tile order.
        for j in range(4):
            nc.sync.dma_start(
                out=utT_sb[32 * j : 32 * j + K, 0:1024],
                in_=utT[32 * j : 32 * j + K, 0:1024],
            )
            nc.gpsimd.dma_start(
                out=utS_sb[32 * j : 32 * j + K, 0:1024],
                in_=utS[32 * j : 32 * j + K, 0:1024],
            )
        for j in range(4):
            nc.sync.dma_start(
                out=utT_sb[32 * j : 32 * j + K, 1024:NPAIR],
                in_=utT[32 * j : 32 * j + K, 1024:NPAIR],
            )
            nc.gpsimd.dma_start(
                out=utS_sb[32 * j : 32 * j + K, 1024:NPAIR],
                in_=utS[32 * j : 32 * j + K, 1024:NPAIR],
            )
        VBLK = 512
        for blk in range(L2H // VBLK):
            for j in range(4):
                nc.scalar.dma_start(
                    out=vt_sb[32 * j : 32 * j + K, blk * VBLK : (blk + 1) * VBLK],
                    in_=vt[32 * j : 32 * j + K, blk * VBLK : (blk + 1) * VBLK],
                )

        mins = stats.tile([128, NSLAB], mybir.dt.float32)

        def mm(dest, s, which, c):
            """Quadrant matmul: slab s=(t,h), chunk c in {0,1}; `which` is the
            utT (start=True) or utS (start=False accumulate) side."""
            t, h = s // 2, s % 2
            q0 = t * 128
            col0 = h * HALF + c * 512
            bp = 32 * (2 * h + c)
            src = utT_sb if which == "T" else utS_sb
            return nc.tensor.matmul(
                dest[:, c * 512 : (c + 1) * 512],
                lhsT=vt_sb[bp : bp + K, q0 : q0 + 128],
                rhs=src[bp : bp + K, col0 : col0 + 512],
                start=(which == "T"),
                stop=True,
                tile_position=(bp, 0),
            )

        slabs = {}

        def mm_T(s):
            P = psum.tile([128, HALF], mybir.dt.float32, tag=f"P{s % 4}")
            slabs[s] = P
            for c in range(2):
                mm(P, s, "T", c)

        def finish_slab(s):
            P = slabs.pop(s)
            relu = nc.scalar.activation(P, P, mybir.ActivationFunctionType.Relu)
            if s + 2 < NSLAB:
                mm_T(s + 2)
            for c in range(2):
                m = mm(P, s, "S", c)
                # start=False is a PSUM read-modify-write; the tracker only
                # sees WAW vs the relu. Force the ordering.
                add_dep_helper(
                    m.ins, relu.ins, sync=True,
                    reason="mm_S accumulates onto ACT-written relu bank",
                )
            nc.vector.tensor_reduce(
                mins[:, s : s + 1], P,
                axis=mybir.AxisListType.X, op=mybir.AluOpType.max,
            )
            if s % FLUSH_EVERY == FLUSH_EVERY - 1:
                nc.sync.dma_start(
                    out=out[:, s - FLUSH_EVERY + 1 : s + 1],
                    in_=mins[:, s - FLUSH_EVERY + 1 : s + 1],
                )

        def body():
            mm_T(0)
            mm_T(1)
            for s in range(NSLAB):
                finish_slab(s)

        if reps == 1:
            body()
        else:
            with tc.For_i(0, reps, 1):
                body()

    nc.finalize()
    return nc


def _split2(x):
    hi = x.astype(BF16)
    lo = (x - hi.astype(np.float32)).astype(BF16)
    return hi, lo


def _split3(x):
    p = x.astype(BF16)
    r1 = x - p.astype(np.float32)
    q = r1.astype(BF16)
    r = (r1 - q.astype(np.float32)).astype(BF16)
    return p, q, r


def _urows(coord, a1):
    """18-row u-side operand block minus the 3 t2-partner rows: coordinate
    content [3, N] (bf16-split twice for the hw/lw v-pairing) and the a1
    scalar row 3-way split."""
    h, lo = _split2(coord)
    p, q, r = _split3(a1[None, :])
    return np.concatenate([h, lo, h, lo, p, q, r], axis=0)  # [15, N]


def _prep_core(c1b, c2b, h):
    """Build vt/utT/utS operands for one core (batch data c1b/c2b, l2 half h)."""
    c2h = c2b[h * L2H : (h + 1) * L2H]  # [L2H, 3]

    # Effective (bf16-split-representable) contour1 coordinates; scalar rows
    # derived from these so the quadratic expansion cancels exactly.
    h1, l1 = _split2(c1b.T)                                   # [3, L1]
    c1eff = h1.astype(np.float64) + l1.astype(np.float64)     # [3, L1]
    s1 = np.sum(c1eff * c1eff, axis=0)
    sum1 = np.sum(c1eff, axis=0)
    a1 = (s1 + 2.0 * EPS * sum1).astype(np.float32)           # [L1]
    c1f = c1eff.astype(np.float32)

    w = -2.0 * c2h.T                                          # [3, L2H]
    hw, lw = _split2(w)
    c2eff = -(hw.astype(np.float64) + lw.astype(np.float64)) / 2.0
    s2 = np.sum(c2eff * c2eff, axis=0)
    sum2 = np.sum(c2eff, axis=0)
    t2 = (s2 - 2.0 * EPS * sum2 + D * EPS * EPS).astype(np.float32)  # [L2H]
    p2, q2, r2 = _split3(t2[None, :])
    vrows = np.concatenate(
        [hw, hw, lw, lw, np.ones((3, L2H), dtype=BF16), p2, q2, r2], axis=0
    )
    assert vrows.shape[0] == K

    # T columns: pair (i, i+NPAIR): effective difference; t2 cancels -> 0 row.
    coordT = (c1f[:, :NPAIR] - c1f[:, NPAIR:]).astype(np.float32)
    a1T = a1[:NPAIR] - a1[NPAIR:]
    uT = np.concatenate(
        [_urows(coordT, a1T), np.zeros((3, NPAIR), dtype=BF16)], axis=0
    )

    # S columns: -d_i -> negate coords, a1, and the t2 partner (-1).
    uS = np.concatenate(
        [_urows(-c1f[:, :NPAIR], -a1[:NPAIR]),
         np.full((3, NPAIR), -1.0, dtype=BF16)], axis=0
    )

    vt = np.zeros((128, L2H), dtype=BF16)
    utT = np.zeros((128, NPAIR), dtype=BF16)
    utS = np.zeros((128, NPAIR), dtype=BF16)
    for j in range(4):
        vt[32 * j : 32 * j + K, :] = vrows
        utT[32 * j : 32 * j + K, :] = uT
        utS[32 * j : 32 * j + K, :] = uS
    return vt, utT, utS


_NC_CACHE = []


def _get_nc():
    if not _NC_CACHE:
        _NC_CACHE.append(_build_nc())
    return _NC_CACHE[0]


def kernel(contour1, contour2, resolution):
    c1 = np.asarray(contour1, dtype=np.float32)
    c2 = np.asarray(contour2, dtype=np.float32)
    res = float(np.asarray(resolution).reshape(-1)[0])

    in_maps = []
    for core in range(NCORES):
        b, h = core // 2, core % 2
        vt, utT, utS = _prep_core(c1[b], c2[b], h)
        in_maps.append({"vt": vt, "utT": utT, "utS": utS})

    nc = _get_nc()
    results = bass_utils.run_bass_kernel_spmd(
        nc, in_maps, core_ids=list(range(NCORES))
    ).results

    out = np.empty((B,), dtype=np.float32)
    for b in range(B):
        halves = []
        for h in range(2):
            r = results[2 * b + h]["out"]  # [128, NSLAB]
            neg_min = np.maximum(r[:, 0::2], r[:, 1::2])  # [128, NT]
            d2 = -neg_min.T.reshape(L2H)
            halves.append(np.sqrt(np.maximum(d2, 0.0)))
        min_dist = np.concatenate(halves)
        out[b] = np.float32(np.mean(min_dist * res))
    return out
